# revision 27
# baseline (speedup 1.0000x reference)
"""Causal self-attention (single head) for Trainium2, 8-core SPMD.

Problem: x [B=4, S=2048, D=1024] fp32; Wq/Wk/Wv [D, D].
  q = x @ Wq.T ; k = x @ Wk.T ; v = x @ Wv.T
  out = softmax(causal(q @ k.T) / sqrt(D)) @ v

Sharding: 8 cores = 4 batches x 2 query shards. Queries are split into
S/128 blocks; shard s of a batch takes blocks g_j = 2j + ((j + s) % 2)
for j in 0..7, which balances causal work: block g attends g+1 key
blocks, and the per-slot padded key-count c_j = 2(j+1) covers both
shards' real counts {2j+1, 2j+2}. Every core runs the identical NEFF;
all per-core differences live in the input data (gathered q columns +
the causal/pad mask for the last 256 key columns of each slot).

Per core: project kT, qT, v from pre-transposed x (host supplies x^T
and swizzled weights so every load is chunk-linear), then per q-slot:
scores = qT.T @ kT (fp32r matmuls, FP22 precision), mask, softmax
along the free dim (ACT exp with accumulated row sum), PE-transpose of
the probability tiles, P.T @ v accumulation, and a final 1/rowsum
scaling. Attention slots are software-pipelined so slot j's softmax
runs under slot j+1's score matmuls.
"""

import sys

for _p in ("/opt/trn_rl_repo", "/root/.axon_site/_ro/trn_rl_repo"):
    if _p not in sys.path:
        sys.path.insert(0, _p)

from contextlib import ExitStack

import numpy as np

import concourse.bass as bass
import concourse.mybir as mybir
import concourse.tile as tile
from concourse import bacc
from concourse.bass_utils import run_bass_kernel_spmd

F32 = mybir.dt.float32
F32R = mybir.dt.float32r
P = 128
MASK_W = 2 * P  # mask covers the last two key blocks of each slot
NEG = -1.0e30


def _ceil_div(a, b):
    return (a + b - 1) // b


def shard_qblocks(shard, nslot):
    """Global 128-row query block ids handled by slot j of this shard."""
    return [2 * j + ((j + shard) % 2) for j in range(nslot)]


def build_nc(D, S, SQ):
    """Build the per-core Bass program.

    D: model dim, S: full seq len (key range), SQ: query rows per core.
    """
    ND = D // P  # contraction chunks
    NE = D // P  # output-feature tiles of qT/kT
    NSLOT = SQ // P  # q slots per core
    NKB = S // P  # key blocks
    KPS = S // NSLOT  # keys added per slot; Kj = KPS*(j+1)
    assert KPS % P == 0 and KPS // P == 2, "slot padding assumes c_j = 2(j+1)"
    CH = min(512, S)  # moving-dim chunk (fp32 PSUM bank = 512)
    QCH = min(512, SQ)
    DCH = min(512, D)
    NEC = D // DCH

    nc = bacc.Bacc("TRN2", target_bir_lowering=False, debug=False, num_devices=8)
    VCH = min(256, S)  # x slab width (linear DMA unit)
    xq_sw = nc.declare_dram_parameter("xq_sw", [SQ // VCH, P, ND, VCH], F32, isOutput=False)
    xkv_sw = nc.declare_dram_parameter("xkv_sw", [S // VCH, P, ND, VCH], F32, isOutput=False)
    wq_sw = nc.declare_dram_parameter("wq_sw", [NE, P, ND, P], F32, isOutput=False)
    wk_sw = nc.declare_dram_parameter("wk_sw", [NE, P, ND, P], F32, isOutput=False)
    wv_sw = nc.declare_dram_parameter("wv_sw", [NEC, P, ND, DCH], F32, isOutput=False)
    mask_in = nc.declare_dram_parameter("mask", [SQ, MASK_W], F32, isOutput=False)
    ident_in = nc.declare_dram_parameter("ident", [P, P], F32, isOutput=False)
    out = nc.declare_dram_parameter("out", [SQ, D], F32, isOutput=True)

    with tile.TileContext(nc) as tc, ExitStack() as ctx:
        # ---- persistent SBUF: kT [NE][P, S], mask, identity (v added later)
        kt_pool = ctx.enter_context(tc.tile_pool(name="kt", bufs=1))
        kt = [kt_pool.tile([P, S], F32R, tag=f"kt{e}", name=f"kt{e}") for e in range(NE)]
        const_pool = ctx.enter_context(tc.tile_pool(name="const", bufs=1))
        ident_sb = const_pool.tile([P, P], F32, tag="ident")
        mask_sb = const_pool.tile([P, NSLOT, MASK_W], F32, tag="mask")
        dram = ctx.enter_context(tc.tile_pool(name="dram", bufs=1, space="DRAM"))
        qt_dram = dram.tile([NSLOT, P, NE * P], F32R)

        # ---- phase K: kT[e, s] = sum_d wkT[d, e] * xkvT[d, s] (stays in SBUF)
        # One shared x-chunk pool (bufs=4) serves K and Q: all four K chunks
        # buffer up front (no reuse waits => no DMA head-of-line blocking),
        # and Q's chunks rotate into slots freed mid-K, so they stream early.
        # wqc lives in the same outer scope so its loads also flow during K.
        q_ctx = ExitStack()
        wq_pool = q_ctx.enter_context(tc.tile_pool(name="wqc", bufs=1))
        wqc = [wq_pool.tile([P, ND, P], F32R, tag=f"wqc{e}", name=f"wqc{e}")
               for e in range(NE)]
        xs_pool = q_ctx.enter_context(tc.tile_pool(name="xs", bufs=4))

        def xs_chunk(src_base, width, engines=None):
            t = xs_pool.tile([P, ND, CH], F32R, tag="xs", name="xs")
            for h in range(width // VCH):
                eng = engines[h % len(engines)] if engines else nc.sync
                eng.dma_start(
                    t[:, :, bass.ts(h, VCH)],
                    src_base[h].bitcast(F32R),
                )
            return t

        with tc.tile_pool(name="wkc", bufs=1) as wk_pool, \
             tc.tile_pool(name="kps", bufs=6, space="PSUM") as kps:
            wkc = [wk_pool.tile([P, ND, P], F32R, tag=f"wkc{e}", name=f"wkc{e}")
                   for e in range(NE)]
            nc.sync.dma_start(wkc[0][:], wk_sw[0].bitcast(F32R))
            nkc = S // CH
            xkcs = [xs_chunk(xkv_sw[0:], CH, engines=[nc.sync, nc.scalar])]
            nc.scalar.dma_start(ident_sb[:], ident_in[:])
            nc.scalar.dma_start(
                mask_sb[:], mask_in.ap().rearrange("(s p) c -> p s c", p=P)
            )
            for et in range(1, NE):
                nc.sync.dma_start(wkc[et][:], wk_sw[et].bitcast(F32R))
            for sc in range(1, nkc):
                xkcs.append(xs_chunk(xkv_sw[sc * (CH // VCH) :], CH))
            for et in range(NE):
                nc.sync.dma_start(wqc[et][:], wq_sw[et].bitcast(F32R))
            for sc in range(nkc):
                xkc = xkcs[sc]
                for et in range(NE):
                    ps = kps.tile([P, CH], F32, tag="kps", name="kps")
                    for d in range(ND):
                        nc.tensor.matmul(
                            ps[:],
                            wkc[et][:, d, :],
                            xkc[:, d, :],
                            start=(d == 0),
                            stop=(d == ND - 1),
                        )
                    nc.vector.tensor_copy(out=kt[et][:, bass.ts(sc, CH)], in_=ps[:])

        # ---- phase Q: qT[e, sq] = sum_d wqT[d, e] * xqT[d, sq] -> DRAM
        # qt_dram is slot-major so the attention readback is one linear DMA.
        with tc.tile_pool(name="qps", bufs=4, space="PSUM") as qps, \
             tc.tile_pool(name="qsb", bufs=3) as qsb:
            for sc in range(SQ // QCH):
                xqc = xs_chunk(xq_sw[sc * (QCH // VCH) :], QCH)
                for et in range(NE):
                    ps = qps.tile([P, QCH], F32, tag="qps", name="qps")
                    for d in range(ND):
                        nc.tensor.matmul(
                            ps[:],
                            wqc[et][:, d, :],
                            xqc[:, d, :QCH],
                            start=(d == 0),
                            stop=(d == ND - 1),
                        )
                    sb = qsb.tile([P, QCH], F32R, tag="qsb", name="qsb")
                    nc.vector.tensor_copy(out=sb[:], in_=ps[:])
                    nj = QCH // P
                    nc.scalar.dma_start(
                        qt_dram[sc * nj : (sc + 1) * nj, :, bass.ts(et, P)]
                        .rearrange("j p q -> p j q"),
                        sb[:].rearrange("p (j q) -> p j q", j=nj),
                    )
        q_ctx.close()  # free wqc + the shared x pool

        # ---- attention pools that must predate phase V (so slot-0/1 score
        # matmuls can run while V's inputs land, hiding the V-entry bubble)
        qt_pool = ctx.enter_context(tc.tile_pool(name="qt", bufs=2))
        sps = ctx.enter_context(tc.tile_pool(name="sps", bufs=4, space="PSUM"))

        NPT = NKB + 2  # rotating P^T slots
        pt_rot = [0]
        sc_tiles = [None] * NSLOT
        vt = []
        att = {}  # filled with the late pools (pt_all, ob, tps, ops)

        held = {}

        def emit_scores_mm(j):
            Kj = KPS * (j + 1)
            qt_j = qt_pool.tile([P, NE * P], F32R, tag="qt", name="qt")
            nc.scalar.dma_start(qt_j[:], qt_dram[j])
            pss = []
            nchunk = _ceil_div(Kj, CH)
            for n in range(nchunk):
                w = min(CH, Kj - CH * n)
                ps = sps.tile([P, CH], F32, tag="sps", name="sps")
                pss.append(ps)
                for e in range(NE):
                    nc.tensor.matmul(
                        ps[:, :w],
                        qt_j[:, bass.ts(e, P)],
                        kt[e][:, CH * n : CH * n + w],
                        start=(e == 0),
                        stop=(e == NE - 1),
                    )
            held[j] = pss

        def emit_scores_fin(j):
            Kj = KPS * (j + 1)
            # stats (negmax / sumexp / rinv) live in the same tile's tail
            sc_j = att["scp"].tile([P, S + 4], F32, tag="sc", name="sc")
            sc_tiles[j] = sc_j
            nchunk = _ceil_div(Kj, CH)
            for n in range(nchunk):
                w = min(CH, Kj - CH * n)
                ps = held[j][n]
                if n == nchunk - 1:
                    if w > MASK_W:
                        nc.vector.tensor_copy(
                            out=sc_j[:, CH * n : Kj - MASK_W],
                            in_=ps[:, : w - MASK_W],
                        )
                    nc.vector.tensor_tensor(
                        sc_j[:, Kj - MASK_W : Kj],
                        ps[:, w - MASK_W : w],
                        mask_sb[:, j, :],
                        mybir.AluOpType.add,
                    )
                else:
                    nc.vector.tensor_copy(
                        out=sc_j[:, bass.ts(n, CH)], in_=ps[:]
                    )
            del held[j]

        def emit_scores(j):
            if j not in held:
                emit_scores_mm(j)
            emit_scores_fin(j)

        def emit_softmax(j):
            Kj = KPS * (j + 1)
            sc_j = sc_tiles[j]
            negmax, sumexp, rinv = sc_j[:, S : S + 1], sc_j[:, S + 1 : S + 2], sc_j[:, S + 2 : S + 3]
            nc.vector.reduce_max(
                negmax, sc_j[:, :Kj], axis=mybir.AxisListType.X, negate=True
            )
            # exp in place over the scores; row-sum accumulated on ACT
            nc.scalar.activation(
                sc_j[:, :Kj],
                sc_j[:, :Kj],
                mybir.ActivationFunctionType.Exp,
                bias=negmax,
                scale=1.0,
                accum_out=sumexp,
            )
            nc.vector.reciprocal(rinv, sumexp)

        def emit_pv(j):
            Kj = KPS * (j + 1)
            cj = Kj // P
            sc_j = sc_tiles[j]
            rinv = sc_j[:, S + 2 : S + 3]
            pts = []
            for kb in range(cj):
                tp = att["tps"].tile([P, P], F32, tag="tps", name="tps")
                nc.tensor.transpose(tp[:], sc_j[:, bass.ts(kb, P)], ident_sb[:])
                pt = att["pt_all"][:, bass.ts(pt_rot[0] % NPT, P)]
                pt_rot[0] += 1
                nc.vector.tensor_copy(out=pt, in_=tp[:])
                pts.append(pt)
            ob = att["obp"].tile([P, D], F32, tag="ob", name="ob")
            for ec in range(D // DCH):
                po = att["ops"].tile([P, DCH], F32, tag="ops", name="ops")
                for kb in range(cj):
                    nc.tensor.matmul(
                        po[:],
                        pts[kb],
                        vt[kb][:, bass.ts(ec, DCH)],
                        start=(kb == 0),
                        stop=(kb == cj - 1),
                    )
                nc.vector.tensor_scalar_mul(
                    ob[:, bass.ts(ec, DCH)], po[:], rinv
                )
            nc.scalar.dma_start(out[bass.ts(j, P), :], ob[:])

        # slot 0/1 score matmuls BEFORE phase V: they need only kt and
        # qt_dram, and keep PE busy across the V pool transition. Results
        # stay parked in PSUM until the sc pool opens after V.
        emit_scores_mm(0)
        if NSLOT > 1:
            emit_scores_mm(1)
        if NSLOT > 2:
            emit_scores_mm(2)

        # ---- phase V: v[s, e] = sum_d xkvT[d, s] * wvT[d, e] (stays in SBUF)
        v_pool = ctx.enter_context(tc.tile_pool(name="v", bufs=1))
        vt.extend(
            v_pool.tile([P, D], F32R, tag=f"v{i}", name=f"v{i}") for i in range(NKB)
        )
        with tc.tile_pool(name="wvc", bufs=1) as wv_pool, \
             tc.tile_pool(name="xvc", bufs=3) as xv_pool, \
             tc.tile_pool(name="vps", bufs=3, space="PSUM") as vps:
            wvc = [wv_pool.tile([P, ND, DCH], F32R, tag=f"wvc{e}", name=f"wvc{e}")
                   for e in range(NEC)]
            xvc0 = xv_pool.tile([P, ND, VCH], F32R, tag="xvc", name="xvc")
            nc.sync.dma_start(xvc0[:], xkv_sw[0].bitcast(F32R))
            nc.scalar.dma_start(wvc[0][:], wv_sw[0].bitcast(F32R))
            for ec in range(1, NEC):
                nc.scalar.dma_start(wvc[ec][:], wv_sw[ec].bitcast(F32R))
            for cv in range(S // VCH):
                if cv == 0:
                    xvc = xvc0
                else:
                    xvc = xv_pool.tile([P, ND, VCH], F32R, tag="xvc", name="xvc")
                    nc.sync.dma_start(xvc[:], xkv_sw[cv].bitcast(F32R))
                for st in range(VCH // P):
                    for ec in range(NEC):
                        ps = vps.tile([P, DCH], F32, tag="vps", name="vps")
                        for d in range(ND):
                            nc.tensor.matmul(
                                ps[:],
                                xvc[:, d, bass.ts(st, P)],
                                wvc[ec][:, d, :],
                                start=(d == 0),
                                stop=(d == ND - 1),
                            )
                        nc.vector.tensor_copy(
                            out=vt[cv * (VCH // P) + st][:, bass.ts(ec, DCH)],
                            in_=ps[:],
                        )

        # ---- attention over the remaining slots (software-pipelined)
        with tc.tile_pool(name="scp", bufs=2) as sc_pool, \
             tc.tile_pool(name="ptp", bufs=1) as pt_pool, \
             tc.tile_pool(name="obp", bufs=2) as ob_pool, \
             tc.tile_pool(name="tps", bufs=2, space="PSUM") as tps, \
             tc.tile_pool(name="ops", bufs=2, space="PSUM") as ops:
            att["scp"] = sc_pool
            att["pt_all"] = pt_pool.tile([P, NPT * P], F32R, tag="pt_all", name="pt_all")
            att["obp"] = ob_pool
            att["tps"] = tps
            att["ops"] = ops
            emit_scores_fin(0)
            emit_softmax(0)
            if NSLOT > 1:
                emit_scores_fin(1)
            for j in range(NSLOT):
                emit_pv(j)
                if j + 1 < NSLOT:
                    emit_softmax(j + 1)
                if j + 2 < NSLOT:
                    emit_scores(j + 2)

    nc.compile()
    return nc


def build_mask(shard, SQ, S):
    """Additive causal/pad mask for the last MASK_W key columns of each slot."""
    nslot = SQ // P
    kps = S // nslot
    gs = shard_qblocks(shard, nslot)
    mask = np.zeros((SQ, MASK_W), dtype=np.float32)
    for j, g in enumerate(gs):
        kj = kps * (j + 1)
        qrow = g * P + np.arange(P)[:, None]  # global query row
        kcol = (kj - MASK_W) + np.arange(MASK_W)[None, :]  # global key col
        mask[j * P : (j + 1) * P] = np.where(kcol <= qrow, 0.0, NEG)
    return mask


def _swizzle_w(wT, D, inner):
    """[D, D] -> [D//inner, P, D//P, inner] so per-tile loads are linear."""
    ND = D // P
    return np.ascontiguousarray(
        wT.reshape(ND, P, D // inner, inner).transpose(2, 1, 0, 3)
    )


_NC_CACHE = {}


def _get_nc(D, S, SQ):
    key = (D, S, SQ)
    if key not in _NC_CACHE:
        _NC_CACHE[key] = build_nc(D, S, SQ)
    return _NC_CACHE[key]


def kernel(x, Wq, Wk, Wv):
    B, S, D = x.shape
    NCORES = 8
    nshard = NCORES // B
    assert nshard == 2, "sharding layout assumes 2 query shards per batch"
    SQ = S // nshard
    nslot = SQ // P
    DCH = min(512, D)

    nc = _get_nc(D, S, SQ)

    VCH = min(256, S)
    ND = D // P

    def swizzle_x(xt):  # [D, S'] -> [S'//VCH, P, ND, VCH]
        sc = xt.shape[1] // VCH
        return np.ascontiguousarray(
            xt.reshape(ND, P, sc, VCH).transpose(2, 1, 0, 3)
        )

    x = np.ascontiguousarray(x, dtype=np.float32)
    xT = np.ascontiguousarray(x.transpose(0, 2, 1))  # [B, D, S]
    scale = np.float32(1.0 / np.sqrt(D))
    wq_sw = _swizzle_w(np.ascontiguousarray(Wq.T.astype(np.float32) * scale), D, P)
    wk_sw = _swizzle_w(np.ascontiguousarray(Wk.T.astype(np.float32)), D, P)
    wv_sw = _swizzle_w(np.ascontiguousarray(Wv.T.astype(np.float32)), D, DCH)
    ident = np.eye(P, dtype=np.float32)
    masks = [build_mask(s, SQ, S) for s in range(nshard)]

    in_maps = []
    for b in range(B):
        for sh in range(nshard):
            gs = shard_qblocks(sh, nslot)
            qidx = np.concatenate([np.arange(g * P, (g + 1) * P) for g in gs])
            in_maps.append(
                {
                    "xq_sw": swizzle_x(xT[b][:, qidx]),
                    "xkv_sw": swizzle_x(xT[b]),
                    "wq_sw": wq_sw,
                    "wk_sw": wk_sw,
                    "wv_sw": wv_sw,
                    "mask": masks[sh],
                    "ident": ident,
                }
            )

    res = run_bass_kernel_spmd(nc, in_maps, list(range(NCORES)))

    out = np.empty((B, S, D), dtype=np.float32)
    c = 0
    for b in range(B):
        for sh in range(nshard):
            o = res.results[c]["out"]
            for j, g in enumerate(shard_qblocks(sh, nslot)):
                out[b, g * P : (g + 1) * P] = o[j * P : (j + 1) * P]
            c += 1
    return out


# revision 28
# speedup vs baseline: 1.0196x; 1.0196x over previous
"""Causal self-attention (single head) for Trainium2, 8-core SPMD.

Problem: x [B=4, S=2048, D=1024] fp32; Wq/Wk/Wv [D, D].
  q = x @ Wq.T ; k = x @ Wk.T ; v = x @ Wv.T
  out = softmax(causal(q @ k.T) / sqrt(D)) @ v

Sharding: 8 cores = 4 batches x 2 query shards. Queries are split into
S/128 blocks; shard s of a batch takes blocks g_j = 2j + ((j + s) % 2)
for j in 0..7, which balances causal work: block g attends g+1 key
blocks, and the per-slot padded key-count c_j = 2(j+1) covers both
shards' real counts {2j+1, 2j+2}. Every core runs the identical NEFF;
all per-core differences live in the input data (gathered q columns +
the causal/pad mask for the last 256 key columns of each slot).

Per core: project kT, qT, v from pre-transposed x (host supplies x^T
and swizzled weights so every load is chunk-linear), then per q-slot:
scores = qT.T @ kT (fp32r matmuls, FP22 precision), mask, softmax
along the free dim (ACT exp with accumulated row sum), PE-transpose of
the probability tiles, P.T @ v accumulation, and a final 1/rowsum
scaling. Attention slots are software-pipelined so slot j's softmax
runs under slot j+1's score matmuls.
"""

import sys

for _p in ("/opt/trn_rl_repo", "/root/.axon_site/_ro/trn_rl_repo"):
    if _p not in sys.path:
        sys.path.insert(0, _p)

from contextlib import ExitStack

import numpy as np

import concourse.bass as bass
import concourse.mybir as mybir
import concourse.tile as tile
from concourse import bacc
from concourse.bass_utils import run_bass_kernel_spmd

F32 = mybir.dt.float32
F32R = mybir.dt.float32r
P = 128
MASK_W = 2 * P  # mask covers the last two key blocks of each slot
NEG = -1.0e30


def _ceil_div(a, b):
    return (a + b - 1) // b


def shard_qblocks(shard, nslot):
    """Global 128-row query block ids handled by slot j of this shard."""
    return [2 * j + ((j + shard) % 2) for j in range(nslot)]


def build_nc(D, S, SQ):
    """Build the per-core Bass program.

    D: model dim, S: full seq len (key range), SQ: query rows per core.
    """
    ND = D // P  # contraction chunks
    NE = D // P  # output-feature tiles of qT/kT
    NSLOT = SQ // P  # q slots per core
    NKB = S // P  # key blocks
    KPS = S // NSLOT  # keys added per slot; Kj = KPS*(j+1)
    assert KPS % P == 0 and KPS // P == 2, "slot padding assumes c_j = 2(j+1)"
    CH = min(512, S)  # moving-dim chunk (fp32 PSUM bank = 512)
    QCH = min(512, SQ)
    DCH = min(512, D)
    NEC = D // DCH

    nc = bacc.Bacc("TRN2", target_bir_lowering=False, debug=False, num_devices=8)
    VCH = min(256, S)  # x slab width (linear DMA unit)
    xq_sw = nc.declare_dram_parameter("xq_sw", [SQ // VCH, P, ND, VCH], F32, isOutput=False)
    xkv_sw = nc.declare_dram_parameter("xkv_sw", [S // VCH, P, ND, VCH], F32, isOutput=False)
    wq_sw = nc.declare_dram_parameter("wq_sw", [NE, P, ND, P], F32, isOutput=False)
    wk_sw = nc.declare_dram_parameter("wk_sw", [NE, P, ND, P], F32, isOutput=False)
    wv_sw = nc.declare_dram_parameter("wv_sw", [NEC, P, ND, DCH], F32, isOutput=False)
    mask_in = nc.declare_dram_parameter("mask", [SQ, MASK_W], F32, isOutput=False)
    ident_in = nc.declare_dram_parameter("ident", [P, P], F32, isOutput=False)
    out = nc.declare_dram_parameter("out", [SQ, D], F32, isOutput=True)

    with tile.TileContext(nc) as tc, ExitStack() as ctx:
        # ---- persistent SBUF: kT [NE][P, S], mask, identity (v added later)
        kt_pool = ctx.enter_context(tc.tile_pool(name="kt", bufs=1))
        kt = [kt_pool.tile([P, S], F32R, tag=f"kt{e}", name=f"kt{e}") for e in range(NE)]
        const_pool = ctx.enter_context(tc.tile_pool(name="const", bufs=1))
        ident_sb = const_pool.tile([P, P], F32, tag="ident")
        mask_sb = const_pool.tile([P, NSLOT, MASK_W], F32, tag="mask")
        dram = ctx.enter_context(tc.tile_pool(name="dram", bufs=1, space="DRAM"))
        qt_dram = dram.tile([NSLOT, P, NE * P], F32R)

        # ---- phase K: kT[e, s] = sum_d wkT[d, e] * xkvT[d, s] (stays in SBUF)
        # One shared x-chunk pool (bufs=4) serves K and Q: all four K chunks
        # buffer up front (no reuse waits => no DMA head-of-line blocking),
        # and Q's chunks rotate into slots freed mid-K, so they stream early.
        # wqc lives in the same outer scope so its loads also flow during K.
        q_ctx = ExitStack()
        wq_pool = q_ctx.enter_context(tc.tile_pool(name="wqc", bufs=1))
        wqc = [wq_pool.tile([P, ND, P], F32R, tag=f"wqc{e}", name=f"wqc{e}")
               for e in range(NE)]
        xs_pool = q_ctx.enter_context(tc.tile_pool(name="xs", bufs=4))

        def xs_chunk(src_base, width, engines=None):
            t = xs_pool.tile([P, ND, CH], F32R, tag="xs", name="xs")
            for h in range(width // VCH):
                eng = engines[h % len(engines)] if engines else nc.sync
                eng.dma_start(
                    t[:, :, bass.ts(h, VCH)],
                    src_base[h].bitcast(F32R),
                )
            return t

        with tc.tile_pool(name="wkc", bufs=1) as wk_pool, \
             tc.tile_pool(name="kps", bufs=6, space="PSUM") as kps:
            wkc = [wk_pool.tile([P, ND, P], F32R, tag=f"wkc{e}", name=f"wkc{e}")
                   for e in range(NE)]
            nc.sync.dma_start(wkc[0][:], wk_sw[0].bitcast(F32R))
            nkc = S // CH
            xkcs = [xs_chunk(xkv_sw[0:], CH, engines=[nc.sync, nc.scalar])]
            nc.scalar.dma_start(ident_sb[:], ident_in[:])
            nc.scalar.dma_start(
                mask_sb[:], mask_in.ap().rearrange("(s p) c -> p s c", p=P)
            )
            for et in range(1, NE):
                nc.sync.dma_start(wkc[et][:], wk_sw[et].bitcast(F32R))
            for sc in range(1, nkc):
                xkcs.append(xs_chunk(xkv_sw[sc * (CH // VCH) :], CH))
            for et in range(NE):
                nc.sync.dma_start(wqc[et][:], wq_sw[et].bitcast(F32R))
            for sc in range(nkc):
                xkc = xkcs[sc]
                for et in range(NE):
                    ps = kps.tile([P, CH], F32, tag="kps", name="kps")
                    for d in range(ND):
                        nc.tensor.matmul(
                            ps[:],
                            wkc[et][:, d, :],
                            xkc[:, d, :],
                            start=(d == 0),
                            stop=(d == ND - 1),
                        )
                    nc.vector.tensor_copy(out=kt[et][:, bass.ts(sc, CH)], in_=ps[:])

        # ---- phase Q: qT[e, sq] = sum_d wqT[d, e] * xqT[d, sq] -> DRAM
        # qt_dram is slot-major so the attention readback is one linear DMA.
        with tc.tile_pool(name="qps", bufs=4, space="PSUM") as qps, \
             tc.tile_pool(name="qsb", bufs=3) as qsb:
            for sc in range(SQ // QCH):
                xqc = xs_chunk(xq_sw[sc * (QCH // VCH) :], QCH)
                for et in range(NE):
                    ps = qps.tile([P, QCH], F32, tag="qps", name="qps")
                    for d in range(ND):
                        nc.tensor.matmul(
                            ps[:],
                            wqc[et][:, d, :],
                            xqc[:, d, :QCH],
                            start=(d == 0),
                            stop=(d == ND - 1),
                        )
                    sb = qsb.tile([P, QCH], F32R, tag="qsb", name="qsb")
                    nc.vector.tensor_copy(out=sb[:], in_=ps[:])
                    nj = QCH // P
                    nc.scalar.dma_start(
                        qt_dram[sc * nj : (sc + 1) * nj, :, bass.ts(et, P)]
                        .rearrange("j p q -> p j q"),
                        sb[:].rearrange("p (j q) -> p j q", j=nj),
                    )
        q_ctx.close()  # free wqc + the shared x pool

        # ---- attention pools that must predate phase V (so slot-0/1 score
        # matmuls can run while V's inputs land, hiding the V-entry bubble)
        qt_pool = ctx.enter_context(tc.tile_pool(name="qt", bufs=2))
        sps = ctx.enter_context(tc.tile_pool(name="sps", bufs=4, space="PSUM"))

        NPT = NKB + 2  # rotating P^T slots
        pt_rot = [0]
        sc_tiles = [None] * NSLOT
        vt = []
        att = {}  # filled with the late pools (pt_all, ob, tps, ops)

        held = {}

        def emit_scores_mm(j):
            Kj = KPS * (j + 1)
            qt_j = qt_pool.tile([P, NE * P], F32R, tag="qt", name="qt")
            nc.scalar.dma_start(qt_j[:], qt_dram[j])
            pss = []
            nchunk = _ceil_div(Kj, CH)
            for n in range(nchunk):
                w = min(CH, Kj - CH * n)
                ps = sps.tile([P, CH], F32, tag="sps", name="sps")
                pss.append(ps)
                for e in range(NE):
                    nc.tensor.matmul(
                        ps[:, :w],
                        qt_j[:, bass.ts(e, P)],
                        kt[e][:, CH * n : CH * n + w],
                        start=(e == 0),
                        stop=(e == NE - 1),
                    )
            held[j] = pss

        def emit_scores_fin(j):
            Kj = KPS * (j + 1)
            # stats (negmax / sumexp / rinv) live in the same tile's tail
            sc_j = att["scp"].tile([P, S + 4], F32, tag="sc", name="sc")
            sc_tiles[j] = sc_j
            nchunk = _ceil_div(Kj, CH)
            for n in range(nchunk):
                w = min(CH, Kj - CH * n)
                ps = held[j][n]
                if n == nchunk - 1:
                    if w > MASK_W:
                        nc.vector.tensor_copy(
                            out=sc_j[:, CH * n : Kj - MASK_W],
                            in_=ps[:, : w - MASK_W],
                        )
                    nc.vector.tensor_tensor(
                        sc_j[:, Kj - MASK_W : Kj],
                        ps[:, w - MASK_W : w],
                        mask_sb[:, j, :],
                        mybir.AluOpType.add,
                    )
                else:
                    nc.vector.tensor_copy(
                        out=sc_j[:, bass.ts(n, CH)], in_=ps[:]
                    )
            del held[j]

        def emit_scores(j):
            if j not in held:
                emit_scores_mm(j)
            emit_scores_fin(j)

        def emit_softmax(j):
            Kj = KPS * (j + 1)
            sc_j = sc_tiles[j]
            negmax, sumexp, rinv = sc_j[:, S : S + 1], sc_j[:, S + 1 : S + 2], sc_j[:, S + 2 : S + 3]
            nc.vector.reduce_max(
                negmax, sc_j[:, :Kj], axis=mybir.AxisListType.X, negate=True
            )
            # exp in place over the scores; row-sum accumulated on ACT
            nc.scalar.activation(
                sc_j[:, :Kj],
                sc_j[:, :Kj],
                mybir.ActivationFunctionType.Exp,
                bias=negmax,
                scale=1.0,
                accum_out=sumexp,
            )
            nc.vector.reciprocal(rinv, sumexp)

        def emit_pv(j):
            Kj = KPS * (j + 1)
            cj = Kj // P
            sc_j = sc_tiles[j]
            rinv = sc_j[:, S + 2 : S + 3]
            pts = []
            for kb in range(cj):
                tp = att["tps"].tile([P, P], F32, tag="tps", name="tps")
                nc.tensor.transpose(tp[:], sc_j[:, bass.ts(kb, P)], ident_sb[:])
                pt = att["pt_all"][:, bass.ts(pt_rot[0] % NPT, P)]
                pt_rot[0] += 1
                nc.vector.tensor_copy(out=pt, in_=tp[:])
                pts.append(pt)
            ob = att["obp"].tile([P, D], F32, tag="ob", name="ob")
            for ec in range(D // DCH):
                po = att["ops"].tile([P, DCH], F32, tag="ops", name="ops")
                for kb in range(cj):
                    nc.tensor.matmul(
                        po[:],
                        pts[kb],
                        vt[kb][:, bass.ts(ec, DCH)],
                        start=(kb == 0),
                        stop=(kb == cj - 1),
                    )
                nc.vector.tensor_scalar_mul(
                    ob[:, bass.ts(ec, DCH)], po[:], rinv
                )
            nc.scalar.dma_start(out[bass.ts(j, P), :], ob[:])

        # slot 0/1 score matmuls BEFORE phase V: they need only kt and
        # qt_dram, and keep PE busy across the V pool transition. Results
        # stay parked in PSUM until the sc pool opens after V.
        emit_scores_mm(0)
        if NSLOT > 1:
            emit_scores_mm(1)
        if NSLOT > 2:
            emit_scores_mm(2)

        # ---- phase V: v[s, e] = sum_d xkvT[d, s] * wvT[d, e] (stays in SBUF)
        v_pool = ctx.enter_context(tc.tile_pool(name="v", bufs=1))
        vt.extend(
            v_pool.tile([P, D], F32R, tag=f"v{i}", name=f"v{i}") for i in range(NKB)
        )
        with tc.tile_pool(name="wvc", bufs=1) as wv_pool, \
             tc.tile_pool(name="xvc", bufs=3) as xv_pool, \
             tc.tile_pool(name="vps", bufs=3, space="PSUM") as vps:
            wvc = [wv_pool.tile([P, ND, DCH], F32R, tag=f"wvc{e}", name=f"wvc{e}")
                   for e in range(NEC)]
            hd = ND // 2
            nc.sync.dma_start(wvc[0][:, :hd], wv_sw[0, :, :hd].bitcast(F32R))
            nc.scalar.dma_start(wvc[0][:, hd:], wv_sw[0, :, hd:].bitcast(F32R))
            xvc0 = xv_pool.tile([P, ND, VCH], F32R, tag="xvc", name="xvc")
            nc.sync.dma_start(xvc0[:], xkv_sw[0].bitcast(F32R))
            for ec in range(1, NEC):
                nc.scalar.dma_start(wvc[ec][:], wv_sw[ec].bitcast(F32R))
            for cv in range(S // VCH):
                if cv == 0:
                    xvc = xvc0
                else:
                    xvc = xv_pool.tile([P, ND, VCH], F32R, tag="xvc", name="xvc")
                    nc.sync.dma_start(xvc[:], xkv_sw[cv].bitcast(F32R))
                for st in range(VCH // P):
                    for ec in range(NEC):
                        ps = vps.tile([P, DCH], F32, tag="vps", name="vps")
                        for d in range(ND):
                            nc.tensor.matmul(
                                ps[:],
                                xvc[:, d, bass.ts(st, P)],
                                wvc[ec][:, d, :],
                                start=(d == 0),
                                stop=(d == ND - 1),
                            )
                        nc.vector.tensor_copy(
                            out=vt[cv * (VCH // P) + st][:, bass.ts(ec, DCH)],
                            in_=ps[:],
                        )

        # ---- attention over the remaining slots (software-pipelined)
        with tc.tile_pool(name="scp", bufs=2) as sc_pool, \
             tc.tile_pool(name="ptp", bufs=1) as pt_pool, \
             tc.tile_pool(name="obp", bufs=2) as ob_pool, \
             tc.tile_pool(name="tps", bufs=2, space="PSUM") as tps, \
             tc.tile_pool(name="ops", bufs=2, space="PSUM") as ops:
            att["scp"] = sc_pool
            att["pt_all"] = pt_pool.tile([P, NPT * P], F32R, tag="pt_all", name="pt_all")
            att["obp"] = ob_pool
            att["tps"] = tps
            att["ops"] = ops
            emit_scores_fin(0)
            emit_softmax(0)
            if NSLOT > 1:
                emit_scores_fin(1)
            for j in range(NSLOT):
                emit_pv(j)
                if j + 1 < NSLOT:
                    emit_softmax(j + 1)
                if j + 2 < NSLOT:
                    emit_scores(j + 2)

    nc.compile()
    return nc


def build_mask(shard, SQ, S):
    """Additive causal/pad mask for the last MASK_W key columns of each slot."""
    nslot = SQ // P
    kps = S // nslot
    gs = shard_qblocks(shard, nslot)
    mask = np.zeros((SQ, MASK_W), dtype=np.float32)
    for j, g in enumerate(gs):
        kj = kps * (j + 1)
        qrow = g * P + np.arange(P)[:, None]  # global query row
        kcol = (kj - MASK_W) + np.arange(MASK_W)[None, :]  # global key col
        mask[j * P : (j + 1) * P] = np.where(kcol <= qrow, 0.0, NEG)
    return mask


def _swizzle_w(wT, D, inner):
    """[D, D] -> [D//inner, P, D//P, inner] so per-tile loads are linear."""
    ND = D // P
    return np.ascontiguousarray(
        wT.reshape(ND, P, D // inner, inner).transpose(2, 1, 0, 3)
    )


_NC_CACHE = {}


def _get_nc(D, S, SQ):
    key = (D, S, SQ)
    if key not in _NC_CACHE:
        _NC_CACHE[key] = build_nc(D, S, SQ)
    return _NC_CACHE[key]


def kernel(x, Wq, Wk, Wv):
    B, S, D = x.shape
    NCORES = 8
    nshard = NCORES // B
    assert nshard == 2, "sharding layout assumes 2 query shards per batch"
    SQ = S // nshard
    nslot = SQ // P
    DCH = min(512, D)

    nc = _get_nc(D, S, SQ)

    VCH = min(256, S)
    ND = D // P

    def swizzle_x(xt):  # [D, S'] -> [S'//VCH, P, ND, VCH]
        sc = xt.shape[1] // VCH
        return np.ascontiguousarray(
            xt.reshape(ND, P, sc, VCH).transpose(2, 1, 0, 3)
        )

    x = np.ascontiguousarray(x, dtype=np.float32)
    xT = np.ascontiguousarray(x.transpose(0, 2, 1))  # [B, D, S]
    scale = np.float32(1.0 / np.sqrt(D))
    wq_sw = _swizzle_w(np.ascontiguousarray(Wq.T.astype(np.float32) * scale), D, P)
    wk_sw = _swizzle_w(np.ascontiguousarray(Wk.T.astype(np.float32)), D, P)
    wv_sw = _swizzle_w(np.ascontiguousarray(Wv.T.astype(np.float32)), D, DCH)
    ident = np.eye(P, dtype=np.float32)
    masks = [build_mask(s, SQ, S) for s in range(nshard)]

    in_maps = []
    for b in range(B):
        for sh in range(nshard):
            gs = shard_qblocks(sh, nslot)
            qidx = np.concatenate([np.arange(g * P, (g + 1) * P) for g in gs])
            in_maps.append(
                {
                    "xq_sw": swizzle_x(xT[b][:, qidx]),
                    "xkv_sw": swizzle_x(xT[b]),
                    "wq_sw": wq_sw,
                    "wk_sw": wk_sw,
                    "wv_sw": wv_sw,
                    "mask": masks[sh],
                    "ident": ident,
                }
            )

    res = run_bass_kernel_spmd(nc, in_maps, list(range(NCORES)))

    out = np.empty((B, S, D), dtype=np.float32)
    c = 0
    for b in range(B):
        for sh in range(nshard):
            o = res.results[c]["out"]
            for j, g in enumerate(shard_qblocks(sh, nslot)):
                out[b, g * P : (g + 1) * P] = o[j * P : (j + 1) * P]
            c += 1
    return out
